# revision 6
# baseline (speedup 1.0000x reference)
"""Trainium2 Bass kernel v2 for nn_Atten_RNN: embedding -> tanh-RNN -> dot-attention -> vocab projection.

Key change vs v1: the recurrent matmul is flipped so the tiny h^T [128,8]
chunks are the stationary operand (weight-load cost ~ columns = 8, near-free)
and W_hh^T streams as the MOVING operand in fp32r (1 cycle/row at N>=512,
vs 4 cyc/row for fp32 and vs 64 full-width LDWEIGHTS reloads per step in v1).
The per-step xW addition is folded into the PSUM accumulation group via an
identity-slice matmul, so no DVE work sits on the serial path.

Sharding: batch-parallel (B=64 -> 8 per core) for RNN + attention; vocab-
parallel (32000 -> 4000 per core) for W_out, joined by one feat AllGather.
"""

import numpy as np
from contextlib import ExitStack

import concourse.bass as bass
import concourse.tile as tile
from concourse import bacc, mybir
from concourse.bass_utils import run_bass_kernel_spmd
from concourse.masks import make_identity

FP = mybir.dt.float32
FPR = mybir.dt.float32r
BF = mybir.dt.bfloat16
I16 = mybir.dt.int16

N_CORES = 8
B = 64
B_LOC = B // N_CORES          # 8
S_FULL = 512
E = 512
H = 1024
V = 32000
V_SH = V // N_CORES           # 4000
KC = H // 128                 # 8 hidden chunks
QC = E // 128                 # 4 embedding chunks
Tanh = mybir.ActivationFunctionType.Tanh
Exp = mybir.ActivationFunctionType.Exp
Copy = mybir.ActivationFunctionType.Copy


def build_nc(S=S_FULL, n_cores=N_CORES, collective=True):
    NT = S * B_LOC            # tokens per core, t = s*B_LOC + b
    ST = S // 128             # attention s-tiles
    TB = NT // 128            # token blocks (xW phase), 16 steps each
    assert S % 128 == 0 and NT % 128 == 0

    nc = bacc.Bacc("TRN2", target_bir_lowering=False, debug=False,
                   num_devices=n_cores)

    # ---- external I/O (per core) ----
    x_idx = nc.dram_tensor("x_idx", [128, NT // 16], I16, kind="ExternalInput")
    emb_t = nc.dram_tensor("emb_t", [V, E], FP, kind="ExternalInput")
    w_ihT = nc.dram_tensor("w_ihT", [QC, 128, H], FP, kind="ExternalInput")
    w_hhT = nc.dram_tensor("w_hhT", [KC, 128, H], FP, kind="ExternalInput")
    bias_row = nc.dram_tensor("bias_row", [1, H], FP, kind="ExternalInput")
    w_outT = nc.dram_tensor("w_outT", [16, 128, V_SH], BF, kind="ExternalInput")
    b_out_sh = nc.dram_tensor("b_out_sh", [1, V_SH], FP, kind="ExternalInput")
    y_out = nc.dram_tensor("y_out", [B, V_SH], FP, kind="ExternalOutput")

    # ---- internal DRAM ----
    # xw_dram[s, b, :] = xW row for token t = s*8+b (plain fp32; consumed by
    # a per-step DVE prefill of the PSUM accumulator, not by a matmul).
    xw_dram = nc.dram_tensor("xw_dram", [S, B_LOC, H], FP)
    out2 = nc.dram_tensor("out2", [S, 128 * KC * B_LOC], FP)   # row s = hT(s) (p, kc, b)
    last_flat = nc.dram_tensor("last_flat", [1, 128 * KC * B_LOC], FP)
    ag_in_l = nc.dram_tensor("ag_in_l", [B_LOC, H], FP)
    ag_out_l = nc.dram_tensor("ag_out_l", [B, H], FP, addr_space="Shared")
    ag_in_a = nc.dram_tensor("ag_in_a", [B_LOC, H], FP)
    ag_out_a = nc.dram_tensor("ag_out_a", [B, H], FP, addr_space="Shared")

    with tile.TileContext(nc) as tc, ExitStack() as top:
        consts = top.enter_context(tc.tile_pool(name="consts", bufs=1))
        ident = consts.tile([128, 128], FP)
        make_identity(nc, ident[:])
        ones_row = consts.tile([1, 128], FP)
        nc.vector.memset(ones_row[:], 1.0)
        bout_sb = consts.tile([1, V_SH], FP)
        nc.sync.dma_start(bout_sb[:], b_out_sh[0:1, :])
        lastT_sb = consts.tile([128, KC * B_LOC], FP)

        # ============ phase 1+2: gather + transpose -> xeT (fp32r) ============
        with tc.tile_pool(name="xeT_p", bufs=1) as xeT_p, \
             tc.tile_pool(name="wih_p", bufs=1) as wih_p:
            wih_r = wih_p.tile([128, QC, H], FPR)
            ones_r = wih_p.tile([1, 128], FPR)
            nc.vector.tensor_copy(ones_r[:], ones_row[:])
            bias_r = wih_p.tile([1, H], FPR)
            xeT_r = xeT_p.tile([128, QC, NT], FPR)
            with tc.tile_pool(name="wih_ld", bufs=1) as wih_ld:
                wih_f = wih_ld.tile([128, QC, H], FP)
                nc.sync.dma_start(wih_f[:], w_ihT.ap().rearrange("q p h -> p q h"))
                nc.vector.tensor_copy(wih_r[:], wih_f[:])
                bias_f = wih_ld.tile([1, H], FP)
                nc.sync.dma_start(bias_f[:], bias_row[0:1, :])
                nc.vector.tensor_copy(bias_r[:], bias_f[:])
            with tc.tile_pool(name="xe_p", bufs=1) as xe_p, \
                 tc.tile_pool(name="idx_p", bufs=1) as idx_p, \
                 tc.tile_pool(name="trp_p", bufs=4, space="PSUM") as trp_p:
                xidx_sb = idx_p.tile([128, NT // 16], I16)
                nc.sync.dma_start(xidx_sb[:], x_idx[:, :])
                xe_sb = xe_p.tile([128, NT // 128, E], FP)
                GCH = 1024  # indices per dma_gather (keeps SWDGE ring within capacity)
                for g in range(NT // GCH):
                    nc.gpsimd.dma_gather(
                        out_ap=xe_sb[:, g * (GCH // 128):(g + 1) * (GCH // 128), :],
                        in_ap=emb_t.ap(),
                        idxs_ap=xidx_sb[:, g * (GCH // 16):(g + 1) * (GCH // 16)],
                        num_idxs=GCH, num_idxs_reg=GCH, elem_size=E)
                for c in range(NT // 128):
                    for q in range(QC):
                        pt = trp_p.tile([128, 128], FP)
                        nc.tensor.transpose(pt[:], xe_sb[:, c, q * 128:(q + 1) * 128], ident[:])
                        nc.vector.tensor_copy(xeT_r[:, q, c * 128:(c + 1) * 128], pt[:])

            # ===== phase 3: xw[t, h] = xe @ W_ih^T + bias, token-major tiles =====
            with tc.tile_pool(name="xw_ps", bufs=4, space="PSUM") as xw_ps, \
                 tc.tile_pool(name="xw_ev", bufs=3) as xw_ev:
                for tb in range(TB):
                    for n in range(2):
                        ps = xw_ps.tile([128, 512], FP, tag="xwps")
                        nc.tensor.matmul(ps[:], ones_r[:, :],
                                         bias_r[0:1, n * 512:(n + 1) * 512],
                                         start=True, stop=False)
                        for q in range(QC):
                            nc.tensor.matmul(
                                ps[:], xeT_r[:, q, tb * 128:(tb + 1) * 128],
                                wih_r[:, q, n * 512:(n + 1) * 512],
                                start=False, stop=(q == QC - 1))
                        ev = xw_ev.tile([128, 512], FP, tag="xwev")
                        nc.vector.tensor_copy(ev[:], ps[:])
                        nc.sync.dma_start(
                            xw_dram[tb * 16:(tb + 1) * 16, :, n * 512:(n + 1) * 512]
                            .rearrange("s b h -> (s b) h"), ev[:])

        # ================= phase 4: RNN (h^T stationary, W_hh moving) =========
        with tc.tile_pool(name="whh_p", bufs=1) as whh_p, \
             tc.tile_pool(name="hT_p", bufs=3) as hT_p, \
             tc.tile_pool(name="hsb_p", bufs=3) as hsb_p, \
             tc.tile_pool(name="hT32_p", bufs=3) as hT32_p, \
             tc.tile_pool(name="xwb_p", bufs=4) as xwb_p, \
             tc.tile_pool(name="pre_ps", bufs=4, space="PSUM") as pre_ps, \
             tc.tile_pool(name="psT_p", bufs=2, space="PSUM") as psT_p:
            whh_r = whh_p.tile([128, KC, H], BF)
            with tc.tile_pool(name="whh_ld", bufs=1) as whh_ld:
                whh_f = whh_ld.tile([128, KC, H], FP)
                nc.sync.dma_start(whh_f[:], w_hhT.ap().rearrange("k p h -> p k h"))
                nc.vector.tensor_copy(whh_r[:], whh_f[:])
            hT_prev = hT_p.tile([128, KC * B_LOC], BF, tag="hT")
            nc.vector.memset(hT_prev[:], 0.0)
            # Software-pipelined loop: T4-7 / CAST-B / out2-DMA of step s-1 are
            # emitted inside iteration s, AFTER step s's first MMs, so the PE
            # never stalls waiting on tanh1(s-1) — next-step chunk-0 matmuls
            # (which only need CAST-A(s-1)) fill the gap.
            HB = KC * B_LOC // 2   # 32: half of the hT column range
            xwb = None
            prev = None            # (psT, hT_new, h_sb, s) of step s-1
            for s in range(S):
                xwb = xwb_p.tile([B_LOC, H], FP, tag="xwb")
                nc.sync.dma_start(xwb[:], xw_dram[s, :, :])
                h_sb = hsb_p.tile([B_LOC, H], FP, tag="hsb")
                # --- chunk 0: DVE xw prefill + W-matmuls c=0..3 (need CAST-A(s-1)) ---
                ps0 = pre_ps.tile([B_LOC, 512], FP, tag="preps")
                nc.vector.tensor_copy(ps0[:], xwb[:, 0:512])
                for c in range(KC // 2):
                    nc.tensor.matmul(ps0[:], hT_prev[:, c * B_LOC:(c + 1) * B_LOC],
                                     whh_r[:, c, 0:512], start=False, stop=False,
                                     skip_group_check=True)
                # --- deferred tail of step s-1: T4-7, CAST-B, out2 ---
                if prev is not None:
                    p_psT, p_hT, p_hsb, p_s = prev
                    for j in range(KC // 2, KC):
                        nc.tensor.transpose(p_psT[:, j * B_LOC:(j + 1) * B_LOC],
                                            p_hsb[:, j * 128:(j + 1) * 128],
                                            ident[0:B_LOC, 0:B_LOC])
                    nc.vector.tensor_copy(p_hT[:, HB:], p_psT[:, HB:])
                    hT32 = hT32_p.tile([128, KC * B_LOC], FP, tag="hT32")
                    nc.scalar.activation(hT32[:], p_psT[:], Copy)
                    nc.sync.dma_start(
                        out2[p_s:p_s + 1, :].rearrange("a (p f) -> (a p) f", p=128),
                        hT32[:])
                # --- chunk 0 rest (c=4..7 need CAST-B(s-1)) + chunk 1 ---
                for c in range(KC // 2, KC):
                    nc.tensor.matmul(ps0[:], hT_prev[:, c * B_LOC:(c + 1) * B_LOC],
                                     whh_r[:, c, 0:512], start=False, stop=(c == KC - 1),
                                     skip_group_check=True)
                nc.scalar.activation(h_sb[:, 0:512], ps0[:], Tanh)
                ps1 = pre_ps.tile([B_LOC, 512], FP, tag="preps")
                nc.vector.tensor_copy(ps1[:], xwb[:, 512:1024])
                for c in range(KC):
                    nc.tensor.matmul(ps1[:], hT_prev[:, c * B_LOC:(c + 1) * B_LOC],
                                     whh_r[:, c, 512:1024], start=False, stop=(c == KC - 1),
                                     skip_group_check=True)
                nc.scalar.activation(h_sb[:, 512:1024], ps1[:], Tanh)
                # --- T0-3 + CAST-A of step s ---
                psT = psT_p.tile([128, KC * B_LOC], FP, tag="psT")
                hT_new = hT_p.tile([128, KC * B_LOC], BF, tag="hT")
                for j in range(KC // 2):
                    nc.tensor.transpose(psT[:, j * B_LOC:(j + 1) * B_LOC],
                                        h_sb[:, j * 128:(j + 1) * 128],
                                        ident[0:B_LOC, 0:B_LOC])
                nc.vector.tensor_copy(hT_new[:, 0:HB], psT[:, 0:HB])
                prev = (psT, hT_new, h_sb, s)
                hT_prev = hT_new
            # final flush for step S-1
            p_psT, p_hT, p_hsb, p_s = prev
            for j in range(KC // 2, KC):
                nc.tensor.transpose(p_psT[:, j * B_LOC:(j + 1) * B_LOC],
                                    p_hsb[:, j * 128:(j + 1) * 128],
                                    ident[0:B_LOC, 0:B_LOC])
            hT32 = hT32_p.tile([128, KC * B_LOC], FP, tag="hT32")
            nc.scalar.activation(hT32[:], p_psT[:], Copy)
            nc.sync.dma_start(
                out2[p_s:p_s + 1, :].rearrange("a (p f) -> (a p) f", p=128), hT32[:])
            nc.vector.tensor_copy(lastT_sb[:], p_psT[:])

        # ================= phase 5: attention =================
        FW = 128 * KC * B_LOC  # 8192: out2 row width
        with tc.tile_pool(name="att_sb", bufs=1) as att_sb:
          with tc.tile_pool(name="lastB_p", bufs=1) as lastB_p, \
               tc.tile_pool(name="att_ps", bufs=2, space="PSUM") as att_ps, \
               tc.tile_pool(name="attacc_ps", bufs=2, space="PSUM") as attacc_ps:
            nc.sync.dma_start(
                last_flat[0:1, :].rearrange("a (p f) -> (a p) f", p=128), lastT_sb[:])
            lastrow_sb = att_sb.tile([1, FW], FP)
            nc.sync.dma_start(lastrow_sb[:], last_flat[0:1, :])
            # early AllGather of the `last` half of feat (hides under attention)
            featT_full = att_sb.tile([128, 16, B], FP)
            feat_l = att_sb.tile([B_LOC, H], FP)
            for i in range(KC):
                fl_ = att_ps.tile([B_LOC, 128], FP, tag="attps")
                nc.tensor.transpose(fl_[:], lastT_sb[:, i * B_LOC:(i + 1) * B_LOC],
                                    ident[:])
                nc.vector.tensor_copy(feat_l[:, i * 128:(i + 1) * 128], fl_[:])
            nc.sync.dma_start(ag_in_l[:, :], feat_l[:])
            if collective:
                nc.gpsimd.collective_compute(
                    "AllGather", mybir.AluOpType.bypass,
                    replica_groups=[list(range(n_cores))],
                    ins=[ag_in_l.ap()], outs=[ag_out_l.ap()])
            else:
                for cc in range(n_cores):
                    nc.sync.dma_start(ag_out_l[cc * B_LOC:(cc + 1) * B_LOC, :],
                                      ag_in_l[:, :])
            featfull_l = att_sb.tile([B, H], FP)
            nc.sync.dma_start(featfull_l[:], ag_out_l[:, :])
            for i in range(KC):
                fq_ = att_ps.tile([128, B], FP, tag="attps")
                nc.tensor.transpose(fq_[:], featfull_l[:, i * 128:(i + 1) * 128],
                                    ident[0:B, 0:B])
                nc.vector.tensor_copy(featT_full[:, KC + i, :], fq_[:])
            lastB = lastB_p.tile([128, FW], FP)
            for i in range(FW // 512):
                lb = att_ps.tile([128, 512], FP, tag="attps")
                nc.tensor.matmul(lb[:], ones_row[:, 0:128],
                                 lastrow_sb[0:1, i * 512:(i + 1) * 512],
                                 start=True, stop=True)
                nc.vector.tensor_copy(lastB[:, i * 512:(i + 1) * 512], lb[:])

            scoresS = att_sb.tile([128, ST * B_LOC], FP)
            with tc.tile_pool(name="o2a_p", bufs=2) as o2a_p, \
                 tc.tile_pool(name="prod_p", bufs=1) as prod_p:
                for c in range(ST):
                    o2t = o2a_p.tile([128, FW], FP, tag="o2a")
                    nc.sync.dma_start(o2t[:], out2[c * 128:(c + 1) * 128, :])
                    pr = prod_p.tile([128, FW], FP, tag="prod")
                    nc.vector.tensor_mul(pr[:], o2t[:], lastB[:])
                    nc.vector.reduce_sum(
                        scoresS[:, c * B_LOC:(c + 1) * B_LOC],
                        pr[:].rearrange("s (hp kc b) -> s b hp kc", hp=128, kc=KC, b=B_LOC),
                        axis=mybir.AxisListType.XY)

            # softmax over time (b on partitions)
            scoresT = att_sb.tile([B_LOC, S], FP)
            for c in range(ST):
                sp = att_ps.tile([B_LOC, 128], FP, tag="attps")
                nc.tensor.transpose(sp[:], scoresS[:, c * B_LOC:(c + 1) * B_LOC], ident[:])
                nc.vector.tensor_copy(scoresT[:, c * 128:(c + 1) * 128], sp[:])
            nc.vector.memset(scoresT[:, S - 1:S], -1e30)  # step S-1 excluded from attention
            negmax = att_sb.tile([B_LOC, 1], FP)
            nc.vector.reduce_max(negmax[:], scoresT[:], axis=mybir.AxisListType.X, negate=True)
            expT = att_sb.tile([B_LOC, S], FP)
            nc.scalar.activation(expT[:], scoresT[:], Exp, bias=negmax[:])
            ssum = att_sb.tile([B_LOC, 1], FP)
            nc.vector.reduce_sum(ssum[:], expT[:], axis=mybir.AxisListType.X)
            rinv = att_sb.tile([B_LOC, 1], FP)
            nc.vector.reciprocal(rinv[:], ssum[:])
            attnT = att_sb.tile([B_LOC, S], FP)
            nc.vector.tensor_scalar_mul(attnT[:], expT[:], rinv[:])
            attnS = att_sb.tile([128, ST, B_LOC], FP)
            for c in range(ST):
                ap_ = att_ps.tile([128, B_LOC], FP, tag="attps")
                nc.tensor.transpose(ap_[:], attnT[:, c * 128:(c + 1) * 128],
                                    ident[0:B_LOC, 0:B_LOC])
                nc.vector.tensor_copy(attnS[:, c, :], ap_[:])

            # att_out^T[h, b] = sum_s attn[s, b] * hT_s[h, b]
            featT = att_sb.tile([128, 2 * KC * B_LOC], FP)
            with tc.tile_pool(name="o2b_p", bufs=2) as o2b_p:
                for c in range(ST):
                    o2t = o2b_p.tile([128, FW], FP, tag="o2b")
                    nc.sync.dma_start(o2t[:], out2[c * 128:(c + 1) * 128, :])
                    o2r = o2t[:].rearrange("s (hp kc b) -> s hp kc b", hp=128, kc=KC, b=B_LOC)
                    pc = attacc_ps.tile([128, KC * B_LOC], FP, tag="attacc")
                    for mc in range(KC):
                        for b in range(B_LOC):
                            nc.tensor.matmul(
                                pc[:, mc * B_LOC + b:mc * B_LOC + b + 1],
                                o2r[:, :, mc, b], attnS[:, c, b:b + 1],
                                start=True, stop=True)
                    if c == 0:
                        nc.vector.tensor_copy(featT[:, 0:KC * B_LOC], pc[:])
                    else:
                        nc.vector.tensor_add(featT[:, 0:KC * B_LOC],
                                             featT[:, 0:KC * B_LOC], pc[:])

            # att half of feat -> AllGather #2 -> featT_full[:, 0:KC, :]
            feat_a = att_sb.tile([B_LOC, H], FP)
            for i in range(KC):
                fp_ = att_ps.tile([B_LOC, 128], FP, tag="attps")
                nc.tensor.transpose(fp_[:], featT[:, i * B_LOC:(i + 1) * B_LOC], ident[:])
                nc.vector.tensor_copy(feat_a[:, i * 128:(i + 1) * 128], fp_[:])
            nc.sync.dma_start(ag_in_a[:, :], feat_a[:])
            if collective:
                nc.gpsimd.collective_compute(
                    "AllGather", mybir.AluOpType.bypass,
                    replica_groups=[list(range(n_cores))],
                    ins=[ag_in_a.ap()], outs=[ag_out_a.ap()])
            else:  # timeline-sim variant: fake the gather with local copies
                for cc in range(n_cores):
                    nc.sync.dma_start(ag_out_a[cc * B_LOC:(cc + 1) * B_LOC, :],
                                      ag_in_a[:, :])
            featfull_a = att_sb.tile([B, H], FP)
            nc.sync.dma_start(featfull_a[:], ag_out_a[:, :])
            for i in range(KC):
                fq = att_ps.tile([128, B], FP, tag="attps")
                nc.tensor.transpose(fq[:], featfull_a[:, i * 128:(i + 1) * 128],
                                    ident[0:B, 0:B])
                nc.vector.tensor_copy(featT_full[:, i, :], fq[:])

          # ================= phase 6: projection =================
          NV = V_SH // 8  # 500-wide psum chunks
          with tc.tile_pool(name="wo_p", bufs=4) as wo_p, \
               tc.tile_pool(name="y_ps", bufs=1, space="PSUM") as y_ps, \
               tc.tile_pool(name="y_sb_p", bufs=1) as y_sb_p:
              psums = [y_ps.tile([B, NV], FP, tag=f"y{n}", name=f"ypsum{n}")
                       for n in range(8)]
              featT_bf = y_sb_p.tile([128, 16, B], BF)
              nc.vector.tensor_copy(featT_bf[:, 8:16, :], featT_full[:, 8:16, :])
              nc.vector.tensor_copy(featT_bf[:, 0:8, :], featT_full[:, 0:8, :])
              for idx, kc in enumerate(list(range(8, 16)) + list(range(8))):
                  wot = wo_p.tile([128, V_SH], BF, tag="wot")
                  nc.sync.dma_start(wot[:], w_outT[kc, :, :])
                  for n in range(8):
                      nc.tensor.matmul(psums[n][:], featT_bf[:, kc, :],
                                       wot[:, n * NV:(n + 1) * NV],
                                       start=(idx == 0), stop=False)
              for n in range(8):
                  nc.tensor.matmul(psums[n][:], ones_row[:, 0:B],
                                   bout_sb[0:1, n * NV:(n + 1) * NV],
                                   start=False, stop=True)
              y_sb = y_sb_p.tile([B, V_SH], FP)
              for n in range(8):
                  nc.vector.tensor_copy(y_sb[:, n * NV:(n + 1) * NV], psums[n][:])
              nc.sync.dma_start(y_out[:, :], y_sb[:])

    nc.compile()
    return nc


def host_prep(X, emb, W_ih, W_hh, b_ih, b_hh, W_out, b_out, S=S_FULL, n_cores=N_CORES):
    """Build the per-core input maps (sharding + layout prep on host)."""
    NT = S * B_LOC
    emb_f = np.ascontiguousarray(np.asarray(emb, np.float32))
    w_ihT = np.ascontiguousarray(
        np.asarray(W_ih, np.float32).T.reshape(QC, 128, H))
    w_hhT = np.ascontiguousarray(
        np.asarray(W_hh, np.float32).T.reshape(KC, 128, H))
    bias_row = np.ascontiguousarray(
        (np.asarray(b_ih, np.float32) + np.asarray(b_hh, np.float32)).reshape(1, H))
    in_maps = []
    for c in range(n_cores):
        Xl = np.asarray(X[c * B_LOC:(c + 1) * B_LOC, :S])
        tok = Xl.T.reshape(-1)                        # t = s*B_LOC + b
        idx = np.zeros((128, NT // 16), np.int16)
        for g in range(8):
            idx[g * 16:(g + 1) * 16, :] = tok.reshape(NT // 16, 16).T
        import ml_dtypes
        Wo = np.asarray(W_out[c * V_SH:(c + 1) * V_SH, :], np.float32)
        w_outT = np.ascontiguousarray(
            Wo.T.reshape(16, 128, V_SH).astype(ml_dtypes.bfloat16))
        in_maps.append({
            "x_idx": idx,
            "emb_t": emb_f,
            "w_ihT": w_ihT,
            "w_hhT": w_hhT,
            "bias_row": bias_row,
            "w_outT": w_outT,
            "b_out_sh": np.asarray(b_out[c * V_SH:(c + 1) * V_SH], np.float32).reshape(1, V_SH),
        })
    return in_maps


_NC_CACHE = {}


def kernel(X, emb, W_ih, W_hh, b_ih, b_hh, W_out, b_out):
    X = np.asarray(X)
    in_maps = host_prep(X, emb, W_ih, W_hh, b_ih, b_hh, W_out, b_out)
    if "nc" not in _NC_CACHE:
        _NC_CACHE["nc"] = build_nc()
    nc = _NC_CACHE["nc"]
    res = run_bass_kernel_spmd(nc, in_maps, list(range(N_CORES)))
    Y = np.concatenate([res.results[i]["y_out"] for i in range(N_CORES)], axis=1)
    return Y.astype(np.float32)


# revision 7
# speedup vs baseline: 26.9099x; 26.9099x over previous
"""Trainium2 Bass kernel v2 for nn_Atten_RNN: embedding -> tanh-RNN -> dot-attention -> vocab projection.

Key change vs v1: the recurrent matmul is flipped so the tiny h^T [128,8]
chunks are the stationary operand (weight-load cost ~ columns = 8, near-free)
and W_hh^T streams as the MOVING operand in fp32r (1 cycle/row at N>=512,
vs 4 cyc/row for fp32 and vs 64 full-width LDWEIGHTS reloads per step in v1).
The per-step xW addition is folded into the PSUM accumulation group via an
identity-slice matmul, so no DVE work sits on the serial path.

Sharding: batch-parallel (B=64 -> 8 per core) for RNN + attention; vocab-
parallel (32000 -> 4000 per core) for W_out, joined by one feat AllGather.
"""

import numpy as np
from contextlib import ExitStack

import concourse.bass as bass
import concourse.tile as tile
from concourse import bacc, mybir
from concourse.bass_utils import run_bass_kernel_spmd
from concourse.masks import make_identity

FP = mybir.dt.float32
FPR = mybir.dt.float32r
BF = mybir.dt.bfloat16
I16 = mybir.dt.int16

N_CORES = 8
B = 64
B_LOC = B // N_CORES          # 8
S_FULL = 512
E = 512
H = 1024
V = 32000
V_SH = V // N_CORES           # 4000
KC = H // 128                 # 8 hidden chunks
QC = E // 128                 # 4 embedding chunks
Tanh = mybir.ActivationFunctionType.Tanh
Exp = mybir.ActivationFunctionType.Exp
Copy = mybir.ActivationFunctionType.Copy


def build_nc(S=S_FULL, n_cores=N_CORES, collective=True):
    NT = S * B_LOC            # tokens per core, t = s*B_LOC + b
    ST = S // 128             # attention s-tiles
    TB = NT // 128            # token blocks (xW phase), 16 steps each
    assert S % 128 == 0 and NT % 128 == 0

    nc = bacc.Bacc("TRN2", target_bir_lowering=False, debug=False,
                   num_devices=n_cores)

    # ---- external I/O (per core) ----
    x_idx = nc.dram_tensor("x_idx", [128, NT // 16], I16, kind="ExternalInput")
    emb_t = nc.dram_tensor("emb_t", [V, E], FP, kind="ExternalInput")
    w_ihT = nc.dram_tensor("w_ihT", [QC, 128, H], FP, kind="ExternalInput")
    w_hhT = nc.dram_tensor("w_hhT", [KC, 128, H], FP, kind="ExternalInput")
    bias_row = nc.dram_tensor("bias_row", [1, H], FP, kind="ExternalInput")
    w_outT = nc.dram_tensor("w_outT", [16, 128, V_SH], BF, kind="ExternalInput")
    b_out_sh = nc.dram_tensor("b_out_sh", [1, V_SH], FP, kind="ExternalInput")
    y_out = nc.dram_tensor("y_out", [B, V_SH], FP, kind="ExternalOutput")

    # ---- internal DRAM ----
    # xw_dram[s, b, :] = xW row for token t = s*8+b (plain fp32; consumed by
    # a per-step DVE prefill of the PSUM accumulator, not by a matmul).
    xw_dram = nc.dram_tensor("xw_dram", [S, B_LOC, H], FP)
    out2 = nc.dram_tensor("out2", [S, 128 * KC * B_LOC], FP)   # row s = hT(s) (p, kc, b)
    last_flat = nc.dram_tensor("last_flat", [1, 128 * KC * B_LOC], FP)
    ag_in = nc.dram_tensor("ag_in", [B_LOC, 2 * H], FP)
    ag_out = nc.dram_tensor("ag_out", [B, 2 * H], FP, addr_space="Shared")

    with tile.TileContext(nc) as tc, ExitStack() as top:
        consts = top.enter_context(tc.tile_pool(name="consts", bufs=1))
        ident = consts.tile([128, 128], FP)
        make_identity(nc, ident[:])
        ones_row = consts.tile([1, 128], FP)
        nc.vector.memset(ones_row[:], 1.0)
        bout_sb = consts.tile([1, V_SH], FP)
        nc.sync.dma_start(bout_sb[:], b_out_sh[0:1, :])
        lastT_sb = consts.tile([128, KC * B_LOC], FP)

        # ============ phase 1+2: gather + transpose -> xeT (fp32r) ============
        with tc.tile_pool(name="xeT_p", bufs=1) as xeT_p, \
             tc.tile_pool(name="wih_p", bufs=1) as wih_p:
            wih_r = wih_p.tile([128, QC, H], FPR)
            ones_r = wih_p.tile([1, 128], FPR)
            nc.vector.tensor_copy(ones_r[:], ones_row[:])
            bias_r = wih_p.tile([1, H], FPR)
            xeT_r = xeT_p.tile([128, QC, NT], FPR)
            with tc.tile_pool(name="wih_ld", bufs=1) as wih_ld:
                wih_f = wih_ld.tile([128, QC, H], FP)
                nc.sync.dma_start(wih_f[:], w_ihT.ap().rearrange("q p h -> p q h"))
                nc.vector.tensor_copy(wih_r[:], wih_f[:])
                bias_f = wih_ld.tile([1, H], FP)
                nc.sync.dma_start(bias_f[:], bias_row[0:1, :])
                nc.vector.tensor_copy(bias_r[:], bias_f[:])
            with tc.tile_pool(name="xe_p", bufs=1) as xe_p, \
                 tc.tile_pool(name="idx_p", bufs=1) as idx_p, \
                 tc.tile_pool(name="trp_p", bufs=4, space="PSUM") as trp_p:
                xidx_sb = idx_p.tile([128, NT // 16], I16)
                nc.sync.dma_start(xidx_sb[:], x_idx[:, :])
                xe_sb = xe_p.tile([128, NT // 128, E], FP)
                GCH = 1024  # indices per dma_gather (keeps SWDGE ring within capacity)
                for g in range(NT // GCH):
                    nc.gpsimd.dma_gather(
                        out_ap=xe_sb[:, g * (GCH // 128):(g + 1) * (GCH // 128), :],
                        in_ap=emb_t.ap(),
                        idxs_ap=xidx_sb[:, g * (GCH // 16):(g + 1) * (GCH // 16)],
                        num_idxs=GCH, num_idxs_reg=GCH, elem_size=E)
                for c in range(NT // 128):
                    for q in range(QC):
                        pt = trp_p.tile([128, 128], FP)
                        nc.tensor.transpose(pt[:], xe_sb[:, c, q * 128:(q + 1) * 128], ident[:])
                        nc.vector.tensor_copy(xeT_r[:, q, c * 128:(c + 1) * 128], pt[:])

            # ===== phase 3: xw[t, h] = xe @ W_ih^T + bias, token-major tiles =====
            with tc.tile_pool(name="xw_ps", bufs=4, space="PSUM") as xw_ps, \
                 tc.tile_pool(name="xw_ev", bufs=3) as xw_ev:
                for tb in range(TB):
                    for n in range(2):
                        ps = xw_ps.tile([128, 512], FP, tag="xwps")
                        nc.tensor.matmul(ps[:], ones_r[:, :],
                                         bias_r[0:1, n * 512:(n + 1) * 512],
                                         start=True, stop=False)
                        for q in range(QC):
                            nc.tensor.matmul(
                                ps[:], xeT_r[:, q, tb * 128:(tb + 1) * 128],
                                wih_r[:, q, n * 512:(n + 1) * 512],
                                start=False, stop=(q == QC - 1))
                        ev = xw_ev.tile([128, 512], FP, tag="xwev")
                        nc.vector.tensor_copy(ev[:], ps[:])
                        nc.sync.dma_start(
                            xw_dram[tb * 16:(tb + 1) * 16, :, n * 512:(n + 1) * 512]
                            .rearrange("s b h -> (s b) h"), ev[:])

        # ================= phase 4: RNN (h^T stationary, W_hh moving) =========
        with tc.tile_pool(name="whh_p", bufs=1) as whh_p, \
             tc.tile_pool(name="hT_p", bufs=3) as hT_p, \
             tc.tile_pool(name="hsb_p", bufs=3) as hsb_p, \
             tc.tile_pool(name="hT32_p", bufs=3) as hT32_p, \
             tc.tile_pool(name="xwb_p", bufs=4) as xwb_p, \
             tc.tile_pool(name="pre_ps", bufs=4, space="PSUM") as pre_ps, \
             tc.tile_pool(name="psT_p", bufs=2, space="PSUM") as psT_p:
            whh_r = whh_p.tile([128, KC, H], BF)
            with tc.tile_pool(name="whh_ld", bufs=1) as whh_ld:
                whh_f = whh_ld.tile([128, KC, H], FP)
                nc.sync.dma_start(whh_f[:], w_hhT.ap().rearrange("k p h -> p k h"))
                nc.vector.tensor_copy(whh_r[:], whh_f[:])
            hT_prev = hT_p.tile([128, KC * B_LOC], BF, tag="hT")
            nc.vector.memset(hT_prev[:], 0.0)
            # Software-pipelined loop: T4-7 / CAST-B / out2-DMA of step s-1 are
            # emitted inside iteration s, AFTER step s's first MMs, so the PE
            # never stalls waiting on tanh1(s-1) — next-step chunk-0 matmuls
            # (which only need CAST-A(s-1)) fill the gap.
            HB = KC * B_LOC // 2   # 32: half of the hT column range
            xwb = None
            prev = None            # (psT, hT_new, h_sb, s) of step s-1
            for s in range(S):
                xwb = xwb_p.tile([B_LOC, H], FP, tag="xwb")
                nc.sync.dma_start(xwb[:], xw_dram[s, :, :])
                h_sb = hsb_p.tile([B_LOC, H], FP, tag="hsb")
                # --- chunk 0: DVE xw prefill + W-matmuls c=0..3 (need CAST-A(s-1)) ---
                ps0 = pre_ps.tile([B_LOC, 512], FP, tag="preps")
                nc.vector.tensor_copy(ps0[:], xwb[:, 0:512])
                for c in range(KC // 2):
                    nc.tensor.matmul(ps0[:], hT_prev[:, c * B_LOC:(c + 1) * B_LOC],
                                     whh_r[:, c, 0:512], start=False, stop=False,
                                     skip_group_check=True)
                # --- deferred tail of step s-1: T4-7, CAST-B, out2 ---
                if prev is not None:
                    p_psT, p_hT, p_hsb, p_s = prev
                    for j in range(KC // 2, KC):
                        nc.tensor.transpose(p_psT[:, j * B_LOC:(j + 1) * B_LOC],
                                            p_hsb[:, j * 128:(j + 1) * 128],
                                            ident[0:B_LOC, 0:B_LOC])
                    nc.vector.tensor_copy(p_hT[:, HB:], p_psT[:, HB:])
                    hT32 = hT32_p.tile([128, KC * B_LOC], FP, tag="hT32")
                    nc.scalar.activation(hT32[:], p_psT[:], Copy)
                    nc.sync.dma_start(
                        out2[p_s:p_s + 1, :].rearrange("a (p f) -> (a p) f", p=128),
                        hT32[:])
                # --- chunk 0 rest (c=4..7 need CAST-B(s-1)) + chunk 1 ---
                for c in range(KC // 2, KC):
                    nc.tensor.matmul(ps0[:], hT_prev[:, c * B_LOC:(c + 1) * B_LOC],
                                     whh_r[:, c, 0:512], start=False, stop=(c == KC - 1),
                                     skip_group_check=True)
                nc.scalar.activation(h_sb[:, 0:512], ps0[:], Tanh)
                ps1 = pre_ps.tile([B_LOC, 512], FP, tag="preps")
                nc.vector.tensor_copy(ps1[:], xwb[:, 512:1024])
                for c in range(KC):
                    nc.tensor.matmul(ps1[:], hT_prev[:, c * B_LOC:(c + 1) * B_LOC],
                                     whh_r[:, c, 512:1024], start=False, stop=(c == KC - 1),
                                     skip_group_check=True)
                nc.scalar.activation(h_sb[:, 512:1024], ps1[:], Tanh)
                # --- T0-3 + CAST-A of step s ---
                psT = psT_p.tile([128, KC * B_LOC], FP, tag="psT")
                hT_new = hT_p.tile([128, KC * B_LOC], BF, tag="hT")
                for j in range(KC // 2):
                    nc.tensor.transpose(psT[:, j * B_LOC:(j + 1) * B_LOC],
                                        h_sb[:, j * 128:(j + 1) * 128],
                                        ident[0:B_LOC, 0:B_LOC])
                nc.vector.tensor_copy(hT_new[:, 0:HB], psT[:, 0:HB])
                prev = (psT, hT_new, h_sb, s)
                hT_prev = hT_new
            # final flush for step S-1
            p_psT, p_hT, p_hsb, p_s = prev
            for j in range(KC // 2, KC):
                nc.tensor.transpose(p_psT[:, j * B_LOC:(j + 1) * B_LOC],
                                    p_hsb[:, j * 128:(j + 1) * 128],
                                    ident[0:B_LOC, 0:B_LOC])
            hT32 = hT32_p.tile([128, KC * B_LOC], FP, tag="hT32")
            nc.scalar.activation(hT32[:], p_psT[:], Copy)
            nc.sync.dma_start(
                out2[p_s:p_s + 1, :].rearrange("a (p f) -> (a p) f", p=128), hT32[:])
            nc.vector.tensor_copy(lastT_sb[:], p_psT[:])

        # ================= phase 5: attention =================
        FW = 128 * KC * B_LOC  # 8192: out2 row width
        with tc.tile_pool(name="att_sb", bufs=1) as att_sb:
          with tc.tile_pool(name="lastB_p", bufs=1) as lastB_p, \
               tc.tile_pool(name="att_ps", bufs=2, space="PSUM") as att_ps, \
               tc.tile_pool(name="attacc_ps", bufs=2, space="PSUM") as attacc_ps:
            nc.sync.dma_start(
                last_flat[0:1, :].rearrange("a (p f) -> (a p) f", p=128), lastT_sb[:])
            lastrow_sb = att_sb.tile([1, FW], FP)
            nc.sync.dma_start(lastrow_sb[:], last_flat[0:1, :])
            lastB = lastB_p.tile([128, FW], FP)
            for i in range(FW // 512):
                lb = att_ps.tile([128, 512], FP, tag="attps")
                nc.tensor.matmul(lb[:], ones_row[:, 0:128],
                                 lastrow_sb[0:1, i * 512:(i + 1) * 512],
                                 start=True, stop=True)
                nc.vector.tensor_copy(lastB[:, i * 512:(i + 1) * 512], lb[:])

            scoresS = att_sb.tile([128, ST * B_LOC], FP)
            with tc.tile_pool(name="o2a_p", bufs=2) as o2a_p, \
                 tc.tile_pool(name="prod_p", bufs=1) as prod_p:
                for c in range(ST):
                    o2t = o2a_p.tile([128, FW], FP, tag="o2a")
                    nc.sync.dma_start(o2t[:], out2[c * 128:(c + 1) * 128, :])
                    pr = prod_p.tile([128, FW], FP, tag="prod")
                    nc.vector.tensor_mul(pr[:], o2t[:], lastB[:])
                    nc.vector.reduce_sum(
                        scoresS[:, c * B_LOC:(c + 1) * B_LOC],
                        pr[:].rearrange("s (hp kc b) -> s b hp kc", hp=128, kc=KC, b=B_LOC),
                        axis=mybir.AxisListType.XY)

            # softmax over time (b on partitions)
            scoresT = att_sb.tile([B_LOC, S], FP)
            for c in range(ST):
                sp = att_ps.tile([B_LOC, 128], FP, tag="attps")
                nc.tensor.transpose(sp[:], scoresS[:, c * B_LOC:(c + 1) * B_LOC], ident[:])
                nc.vector.tensor_copy(scoresT[:, c * 128:(c + 1) * 128], sp[:])
            nc.vector.memset(scoresT[:, S - 1:S], -1e30)  # step S-1 excluded from attention
            negmax = att_sb.tile([B_LOC, 1], FP)
            nc.vector.reduce_max(negmax[:], scoresT[:], axis=mybir.AxisListType.X, negate=True)
            expT = att_sb.tile([B_LOC, S], FP)
            nc.scalar.activation(expT[:], scoresT[:], Exp, bias=negmax[:])
            ssum = att_sb.tile([B_LOC, 1], FP)
            nc.vector.reduce_sum(ssum[:], expT[:], axis=mybir.AxisListType.X)
            rinv = att_sb.tile([B_LOC, 1], FP)
            nc.vector.reciprocal(rinv[:], ssum[:])
            attnT = att_sb.tile([B_LOC, S], FP)
            nc.vector.tensor_scalar_mul(attnT[:], expT[:], rinv[:])
            attnS = att_sb.tile([128, ST, B_LOC], FP)
            for c in range(ST):
                ap_ = att_ps.tile([128, B_LOC], FP, tag="attps")
                nc.tensor.transpose(ap_[:], attnT[:, c * 128:(c + 1) * 128],
                                    ident[0:B_LOC, 0:B_LOC])
                nc.vector.tensor_copy(attnS[:, c, :], ap_[:])

            # att_out^T[h, b] = sum_s attn[s, b] * hT_s[h, b]
            featT = att_sb.tile([128, 2 * KC * B_LOC], FP)
            with tc.tile_pool(name="o2b_p", bufs=2) as o2b_p:
                for c in range(ST):
                    o2t = o2b_p.tile([128, FW], FP, tag="o2b")
                    nc.sync.dma_start(o2t[:], out2[c * 128:(c + 1) * 128, :])
                    o2r = o2t[:].rearrange("s (hp kc b) -> s hp kc b", hp=128, kc=KC, b=B_LOC)
                    pc = attacc_ps.tile([128, KC * B_LOC], FP, tag="attacc")
                    for mc in range(KC):
                        for b in range(B_LOC):
                            nc.tensor.matmul(
                                pc[:, mc * B_LOC + b:mc * B_LOC + b + 1],
                                o2r[:, :, mc, b], attnS[:, c, b:b + 1],
                                start=True, stop=True)
                    if c == 0:
                        nc.vector.tensor_copy(featT[:, 0:KC * B_LOC], pc[:])
                    else:
                        nc.vector.tensor_add(featT[:, 0:KC * B_LOC],
                                             featT[:, 0:KC * B_LOC], pc[:])

            # featT = [att_out^T ; last^T]  -> feat rows -> AllGather -> featT_full
            nc.vector.tensor_copy(featT[:, KC * B_LOC:], lastT_sb[:])
            feat_sb = att_sb.tile([B_LOC, 2 * H], FP)
            for i in range(2 * KC):
                fp_ = att_ps.tile([B_LOC, 128], FP, tag="attps")
                nc.tensor.transpose(fp_[:], featT[:, i * B_LOC:(i + 1) * B_LOC], ident[:])
                nc.vector.tensor_copy(feat_sb[:, i * 128:(i + 1) * 128], fp_[:])
            nc.sync.dma_start(ag_in[:, :], feat_sb[:])
            if collective:
                nc.gpsimd.collective_compute(
                    "AllGather", mybir.AluOpType.bypass,
                    replica_groups=[list(range(n_cores))],
                    ins=[ag_in.ap()], outs=[ag_out.ap()])
            else:  # timeline-sim variant: fake the gather with local copies
                for cc in range(n_cores):
                    nc.sync.dma_start(ag_out[cc * B_LOC:(cc + 1) * B_LOC, :], ag_in[:, :])
            featfull = att_sb.tile([B, 2 * H], FP)
            nc.sync.dma_start(featfull[:], ag_out[:, :])
            featT_full = att_sb.tile([128, 16, B], FP)
            for i in range(16):
                fq = att_ps.tile([128, B], FP, tag="attps")
                nc.tensor.transpose(fq[:], featfull[:, i * 128:(i + 1) * 128], ident[0:B, 0:B])
                nc.vector.tensor_copy(featT_full[:, i, :], fq[:])

          # ================= phase 6: projection =================
          NV = V_SH // 8  # 500-wide psum chunks
          with tc.tile_pool(name="wo_p", bufs=4) as wo_p, \
               tc.tile_pool(name="y_ps", bufs=1, space="PSUM") as y_ps, \
               tc.tile_pool(name="y_sb_p", bufs=1) as y_sb_p:
              psums = [y_ps.tile([B, NV], FP, tag=f"y{n}", name=f"ypsum{n}")
                       for n in range(8)]
              featT_bf = y_sb_p.tile([128, 16, B], BF)
              nc.vector.tensor_copy(featT_bf[:], featT_full[:])
              for kc in range(16):
                  wot = wo_p.tile([128, V_SH], BF, tag="wot")
                  nc.sync.dma_start(wot[:], w_outT[kc, :, :])
                  for n in range(8):
                      nc.tensor.matmul(psums[n][:], featT_bf[:, kc, :],
                                       wot[:, n * NV:(n + 1) * NV],
                                       start=(kc == 0), stop=False)
              for n in range(8):
                  nc.tensor.matmul(psums[n][:], ones_row[:, 0:B],
                                   bout_sb[0:1, n * NV:(n + 1) * NV],
                                   start=False, stop=True)
              y_sb = y_sb_p.tile([B, V_SH], FP)
              for n in range(8):
                  nc.vector.tensor_copy(y_sb[:, n * NV:(n + 1) * NV], psums[n][:])
              nc.sync.dma_start(y_out[:, :], y_sb[:])

    nc.compile()
    return nc


def host_prep(X, emb, W_ih, W_hh, b_ih, b_hh, W_out, b_out, S=S_FULL, n_cores=N_CORES):
    """Build the per-core input maps (sharding + layout prep on host)."""
    NT = S * B_LOC
    emb_f = np.ascontiguousarray(np.asarray(emb, np.float32))
    w_ihT = np.ascontiguousarray(
        np.asarray(W_ih, np.float32).T.reshape(QC, 128, H))
    w_hhT = np.ascontiguousarray(
        np.asarray(W_hh, np.float32).T.reshape(KC, 128, H))
    bias_row = np.ascontiguousarray(
        (np.asarray(b_ih, np.float32) + np.asarray(b_hh, np.float32)).reshape(1, H))
    in_maps = []
    for c in range(n_cores):
        Xl = np.asarray(X[c * B_LOC:(c + 1) * B_LOC, :S])
        tok = Xl.T.reshape(-1)                        # t = s*B_LOC + b
        idx = np.zeros((128, NT // 16), np.int16)
        for g in range(8):
            idx[g * 16:(g + 1) * 16, :] = tok.reshape(NT // 16, 16).T
        import ml_dtypes
        Wo = np.asarray(W_out[c * V_SH:(c + 1) * V_SH, :], np.float32)
        w_outT = np.ascontiguousarray(
            Wo.T.reshape(16, 128, V_SH).astype(ml_dtypes.bfloat16))
        in_maps.append({
            "x_idx": idx,
            "emb_t": emb_f,
            "w_ihT": w_ihT,
            "w_hhT": w_hhT,
            "bias_row": bias_row,
            "w_outT": w_outT,
            "b_out_sh": np.asarray(b_out[c * V_SH:(c + 1) * V_SH], np.float32).reshape(1, V_SH),
        })
    return in_maps


_NC_CACHE = {}


def kernel(X, emb, W_ih, W_hh, b_ih, b_hh, W_out, b_out):
    X = np.asarray(X)
    in_maps = host_prep(X, emb, W_ih, W_hh, b_ih, b_hh, W_out, b_out)
    if "nc" not in _NC_CACHE:
        _NC_CACHE["nc"] = build_nc()
    nc = _NC_CACHE["nc"]
    res = run_bass_kernel_spmd(nc, in_maps, list(range(N_CORES)))
    Y = np.concatenate([res.results[i]["y_out"] for i in range(N_CORES)], axis=1)
    return Y.astype(np.float32)
